# revision 7
# baseline (speedup 1.0000x reference)
"""AttentionalPropagation (SuperGlue-style GNN message passing) on 8 trn2 cores.

Problem (hardcoded): B=2, D=256, N=M=4096, H=4 heads, head dim 64.
  q = P_q(x); k = P_k(source); v = P_v(source)      (bottleneck 1x1 convs D->D/8->D)
  msg = attn(q, k, v); merged = P_m(msg)            (per-head softmax over M)
  out = Conv(relu(BN(Conv(cat[x, merged]))))        (512->64->256)

Sharding: 8 cores = (batch b in {0,1}) x (query chunk of 1024).  Each core
computes k/v for its full batch row (cheap, duplicated 4x) and attention +
MLP for its 1024 query columns.  Weights replicated.  No collectives.

Layout trick: everything stays channels-on-partitions except attention scores,
which are computed transposed (keys m on partitions, queries n free) so that
softmax normalization folds into the PE: the value matrix vT carries an extra
all-ones column per head, making row 64 of the msg-PSUM the softmax
denominator.  Head channels are made contiguous by permuting weight rows/cols
on the host.  Big matmuls run as float32r (full PE rate; fp32 is 4x slower).
"""

import numpy as np

import concourse.bass as bass
import concourse.mybir as mybir
import concourse.tile as tile
from concourse import bacc, bass_utils

B, D, N, M, H = 2, 256, 4096, 4096, 4
DIM = D // H       # 64
D8 = D // 8        # 32
TD = 2 * D         # 512
TD8 = TD // 8      # 64
BN_EPS = 1e-5
NCORES = 8
NCHUNK = N // 4    # query columns per core
NT = 512           # n tile (fp32 matmul moving-operand / PSUM bank limit)
NTILES = NCHUNK // NT          # 2
MT = 512           # source m tile for k/v projection stage
MTILES = M // MT               # 8
MC = 128           # m chunk (scores PSUM partition dim)
MCHUNKS = M // MC              # 32
F32 = mybir.dt.float32
F32R = mybir.dt.float32r
AF = mybir.ActivationFunctionType


def _mm(nc, out, lhsT, rhs, start, stop):
    nc.tensor.matmul(out, lhsT, rhs, start=start, stop=stop)


def build_body(ctx, tc: tile.TileContext, io):
    nc = tc.nc
    x_d = io["x_chunk"]          # [2, 128, NCHUNK]  (channel-chunk, partition, n)
    src_d = io["source_b"]       # [2, 128, M]
    out_d = io["out_chunk"]      # [2, 128, NCHUNK]

    consts = ctx.enter_context(tc.tile_pool(name="consts", bufs=1))
    big = ctx.enter_context(tc.tile_pool(name="big", bufs=1))
    srcp = ctx.enter_context(tc.tile_pool(name="srcp", bufs=3))
    kv1p = ctx.enter_context(tc.tile_pool(name="kv1p", bufs=3))
    ep = ctx.enter_context(tc.tile_pool(name="ep", bufs=6))
    nrm = ctx.enter_context(tc.tile_pool(name="nrm", bufs=4))
    pp = ctx.enter_context(tc.tile_pool(name="pp", bufs=2, space="PSUM"))
    pps = ctx.enter_context(tc.tile_pool(name="pps", bufs=4, space="PSUM"))
    ppm = ctx.enter_context(tc.tile_pool(name="ppm", bufs=2, space="PSUM"))

    # ---- constants / weights ----
    def wtile(name, shape, dt=F32R):
        t = consts.tile(shape, dt, name=name)
        nc.sync.dma_start(out=t, in_=io[name])
        return t

    wq1t = wtile("wq1t", [128, 2, D8])
    wk1t = wtile("wk1t", [128, 2, D8])
    wv1t = wtile("wv1t", [128, 2, D8])
    bq1 = wtile("bq1", [D8, 1], F32)
    bk1 = wtile("bk1", [D8, 1], F32)
    bv1 = wtile("bv1", [D8, 1], F32)
    wq2t = wtile("wq2t", [D8 + 1, 2, 128])
    wk2t = wtile("wk2t", [D8 + 1, 2, 128])
    rv = wtile("rv", [D8 + 1, H * (DIM + 1)])
    wm1t = wtile("wm1t", [DIM, H, D8])
    bm1 = wtile("bm1", [1, D8])
    wm2t = wtile("wm2t", [D8 + 1, 2, 128])
    wp1xt = wtile("wp1xt", [128, 2, TD8])
    wp1mt = wtile("wp1mt", [128, 2, TD8])
    bp1 = wtile("bp1", [1, TD8])
    g1s = wtile("g1s", [TD8, 1], F32)
    be1 = wtile("be1", [TD8, 1], F32)
    wp2t = wtile("wp2t", [TD8 + 1, 2, 128])

    ones_row = wtile("ones", [1, NCHUNK])

    # ---- persistent activations ----
    x_sb = big.tile([128, 2, NCHUNK], F32R)
    for ct in range(2):
        nc.sync.dma_start(out=x_sb[:, ct, :], in_=x_d[ct])
    k_sb = big.tile([128, 2, M], F32R)            # k', heads contiguous
    vT_sb = big.tile([128, MCHUNKS, H * (DIM + 1)], F32R)
    q_sb = big.tile([128, 2, NCHUNK], F32R)
    msg_sb = big.tile([DIM, H, NCHUNK], F32R)

    # ---- k / v projections (full M, streamed in m tiles) ----
    for mt in range(MTILES):
        ms = mt * MT
        src = srcp.tile([128, 2, MT], F32R, tag="src")
        for ct in range(2):
            nc.sync.dma_start(out=src[:, ct, :], in_=src_d[ct, :, ms:ms + MT])
        k1 = kv1p.tile([D8 + 1, MT], F32R, tag="k1")
        v1 = kv1p.tile([D8 + 1, MT], F32R, tag="v1")
        for (w1, b1, t1) in ((wk1t, bk1, k1), (wv1t, bv1, v1)):
            ps1 = pp.tile([D8, MT], F32, tag="pp", name="ps1")
            _mm(nc, ps1, w1[:, 0, :], src[:, 0, :], True, False)
            _mm(nc, ps1, w1[:, 1, :], src[:, 1, :], False, True)
            nc.vector.tensor_scalar_add(out=t1[0:D8, :], in0=ps1, scalar1=b1)
            nc.vector.tensor_copy(out=t1[D8:D8 + 1, :], in_=ones_row[:, 0:MT])
        for ct in range(2):
            psk = pp.tile([128, MT], F32, tag="pp", name="psk")
            _mm(nc, psk, wk2t[:, ct, :], k1, True, True)
            nc.vector.tensor_copy(out=k_sb[:, ct, ms:ms + MT], in_=psk)
        for j in range(MT // MC):
            mc = (ms // MC) + j
            psv = pp.tile([128, H * (DIM + 1)], F32, tag="pp", name="psv")
            _mm(nc, psv, v1[:, j * MC:(j + 1) * MC], rv, True, True)
            nc.vector.tensor_copy(out=vT_sb[:, mc, :], in_=psv)

    # ---- q projection (this core's n chunk) ----
    q1 = big.tile([D8 + 1, NCHUNK], F32R)
    for nt in range(NTILES):
        ns = nt * NT
        psq = pp.tile([D8, NT], F32, tag="pp", name="psq")
        _mm(nc, psq, wq1t[:, 0, :], x_sb[:, 0, ns:ns + NT], True, False)
        _mm(nc, psq, wq1t[:, 1, :], x_sb[:, 1, ns:ns + NT], False, True)
        nc.vector.tensor_scalar_add(out=q1[0:D8, ns:ns + NT], in0=psq, scalar1=bq1)
    nc.vector.tensor_copy(out=q1[D8:D8 + 1, :], in_=ones_row)
    for ct in range(2):
        for nt in range(NTILES):
            ns = nt * NT
            psq2 = pp.tile([128, NT], F32, tag="pp", name="psq2")
            _mm(nc, psq2, wq2t[:, ct, :], q1[:, ns:ns + NT], True, True)
            nc.vector.tensor_copy(out=q_sb[:, ct, ns:ns + NT], in_=psq2)

    # ---- attention ----
    # scores^T chunk: [m=128, n=NT] = k_h[:, mchunk].T @ q_h[:, ntile]
    # exp via ACT (scale folds 1/sqrt(DIM)); msg psum accumulates vT' @ exp
    # where vT' col 64 per head is all-ones -> psum row 64 = softmax denom.
    for nt in range(NTILES):
        ns = nt * NT
        for pair in range(2):          # heads (2*pair, 2*pair+1) row-packed on PE
            pm0 = ppm.tile([DIM + 1, NT], F32, tag="pm", name="pm0")
            pm1 = ppm.tile([DIM + 1, NT], F32, tag="pm", name="pm1")
            for mc in range(MCHUNKS):
                ps0 = pps.tile([128, NT], F32, tag="ps", name="ps0")
                ps1 = pps.tile([128, NT], F32, tag="ps", name="ps1")
                _mm(nc, ps0, k_sb[0:64, pair, mc * MC:(mc + 1) * MC],
                    q_sb[0:64, pair, ns:ns + NT], True, True)
                _mm(nc, ps1, k_sb[64:128, pair, mc * MC:(mc + 1) * MC],
                    q_sb[64:128, pair, ns:ns + NT], True, True)
                e0 = ep.tile([128, NT], F32R, tag="e", name="e0")
                e1 = ep.tile([128, NT], F32R, tag="e", name="e1")
                nc.scalar.activation(out=e0, in_=ps0, func=AF.Exp, scale=0.125)
                nc.scalar.activation(out=e1, in_=ps1, func=AF.Exp, scale=0.125)
                st, sp = mc == 0, mc == MCHUNKS - 1
                h0, h1 = 2 * pair, 2 * pair + 1
                _mm(nc, pm0, vT_sb[:, mc, h0 * 65:h0 * 65 + 65], e0, st, sp)
                _mm(nc, pm1, vT_sb[:, mc, h1 * 65:h1 * 65 + 65], e1, st, sp)
            for hh, pm in ((2 * pair, pm0), (2 * pair + 1, pm1)):
                rec = nrm.tile([1, NT], F32, tag="rec", name="rec")
                nc.vector.reciprocal(out=rec, in_=pm[DIM:DIM + 1, :])
                bc = nrm.tile([DIM, NT], F32, tag="bc", name="bc")
                nc.gpsimd.partition_broadcast(bc, rec)
                nc.vector.tensor_mul(out=msg_sb[:, hh, ns:ns + NT],
                                     in0=pm[0:DIM, :], in1=bc)

    # ---- merge projection ----
    m1 = big.tile([D8 + 1, NCHUNK], F32R)
    for nt in range(NTILES):
        ns = nt * NT
        psm = pp.tile([D8, NT], F32, tag="pp", name="psm")
        for h in range(H):
            _mm(nc, psm, wm1t[:, h, :], msg_sb[:, h, ns:ns + NT], h == 0, False)
        _mm(nc, psm, bm1, ones_row[:, 0:NT], False, True)
        nc.vector.tensor_copy(out=m1[0:D8, ns:ns + NT], in_=psm)
    nc.vector.tensor_copy(out=m1[D8:D8 + 1, :], in_=ones_row)
    mm_sb = big.tile([128, 2, NCHUNK], F32R)      # merged msg, unpermuted chans
    for ct in range(2):
        for nt in range(NTILES):
            ns = nt * NT
            psm2 = pp.tile([128, NT], F32, tag="pp", name="psm2")
            _mm(nc, psm2, wm2t[:, ct, :], m1[:, ns:ns + NT], True, True)
            nc.vector.tensor_copy(out=mm_sb[:, ct, ns:ns + NT], in_=psm2)

    # ---- MLP: conv(cat[x, msg]) -> BN(eval) -> relu -> conv ----
    h1 = big.tile([TD8 + 1, NCHUNK], F32R)
    for nt in range(NTILES):
        ns = nt * NT
        psh = pp.tile([TD8, NT], F32, tag="pp", name="psh")
        _mm(nc, psh, wp1xt[:, 0, :], x_sb[:, 0, ns:ns + NT], True, False)
        _mm(nc, psh, wp1xt[:, 1, :], x_sb[:, 1, ns:ns + NT], False, False)
        _mm(nc, psh, wp1mt[:, 0, :], mm_sb[:, 0, ns:ns + NT], False, False)
        _mm(nc, psh, wp1mt[:, 1, :], mm_sb[:, 1, ns:ns + NT], False, False)
        _mm(nc, psh, bp1, ones_row[:, 0:NT], False, True)
        nc.scalar.activation(out=h1[0:TD8, ns:ns + NT], in_=psh, func=AF.Relu,
                             bias=be1, scale=g1s)
    nc.vector.tensor_copy(out=h1[TD8:TD8 + 1, :], in_=ones_row)
    out_sb = big.tile([128, 2, NCHUNK], F32)
    for ct in range(2):
        for nt in range(NTILES):
            ns = nt * NT
            pso = pp.tile([128, NT], F32, tag="pp", name="pso")
            _mm(nc, pso, wp2t[:, ct, :], h1[:, ns:ns + NT], True, True)
            nc.vector.tensor_copy(out=out_sb[:, ct, ns:ns + NT], in_=pso)
        nc.sync.dma_start(out=out_d[ct], in_=out_sb[:, ct, :])


def build_program():
    nc = bacc.Bacc("TRN2", target_bir_lowering=False, debug=False)
    io = {}
    def inp(name, shape, dt=F32R):
        io[name] = nc.dram_tensor(name, shape, dt, kind="ExternalInput").ap()
    inp("x_chunk", [2, 128, NCHUNK])
    inp("source_b", [2, 128, M])
    inp("wq1t", [128, 2, D8]); inp("bq1", [D8, 1], F32)
    inp("wk1t", [128, 2, D8]); inp("bk1", [D8, 1], F32)
    inp("wv1t", [128, 2, D8]); inp("bv1", [D8, 1], F32)
    inp("wq2t", [D8 + 1, 2, 128])
    inp("wk2t", [D8 + 1, 2, 128])
    inp("rv", [D8 + 1, H * (DIM + 1)])
    inp("wm1t", [DIM, H, D8]); inp("bm1", [1, D8])
    inp("wm2t", [D8 + 1, 2, 128])
    inp("wp1xt", [128, 2, TD8]); inp("wp1mt", [128, 2, TD8]); inp("bp1", [1, TD8])
    inp("g1s", [TD8, 1], F32); inp("be1", [TD8, 1], F32)
    inp("wp2t", [TD8 + 1, 2, 128])
    inp("ones", [1, NCHUNK])
    io["out_chunk"] = nc.dram_tensor(
        "out_chunk", [2, 128, NCHUNK], F32, kind="ExternalOutput").ap()
    from contextlib import ExitStack
    with tile.TileContext(nc) as tc, ExitStack() as ctx:
        build_body(ctx, tc, io)
    nc.compile()
    return nc


def prep_weights(i):
    kernel_nchunk = NCHUNK
    """Host-side preprocessing: transposes, head-channel permutation, bias
    folding (extra contraction rows), BN folding."""
    f = np.float32
    a = {k: np.asarray(v, dtype=f) for k, v in i.items()}
    # permutation making head channels contiguous: c' = h*64+d  <- c = 4*d+h
    perm = (np.arange(H)[:, None] + H * np.arange(DIM)[None, :]).reshape(-1)

    def w1t(w):       # [D8, D] -> [128, 2, D8]
        return np.ascontiguousarray(w.T.reshape(2, 128, D8).swapaxes(0, 1))

    def w2t(w, b):    # [D, D8] x [D] -> [D8+1, 2, 128], rows = [w.T; b]
        top = w.T.reshape(D8, 2, 128)
        return np.ascontiguousarray(np.concatenate([top, b.reshape(1, 2, 128)], 0))

    out = {
        "wq1t": w1t(a["Wq1"]), "bq1": a["bq1"].reshape(D8, 1),
        "wk1t": w1t(a["Wk1"]), "bk1": a["bk1"].reshape(D8, 1),
        "wv1t": w1t(a["Wv1"]), "bv1": a["bv1"].reshape(D8, 1),
        "wq2t": w2t(a["Wq2"][perm], a["bq2"][perm]),
        "wk2t": w2t(a["Wk2"][perm], a["bk2"][perm]),
        "wm2t": w2t(a["Wm2"], a["bm2"]),
        "wp2t": np.ascontiguousarray(np.concatenate(
            [a["Wp2"].T.reshape(TD8, 2, 128), a["bp2"].reshape(1, 2, 128)], 0)),
        "bm1": a["bm1"].reshape(1, D8),
        "bp1": a["bp1"].reshape(1, TD8),
        "g1s": (a["g1"] / np.sqrt(f(1.0) + f(BN_EPS))).reshape(TD8, 1).astype(f),
        "be1": a["be1"].reshape(TD8, 1),
        "ones": np.ones((1, kernel_nchunk), f),
    }
    # rv: [33, 260]; per head h: cols [65h:65h+64] = [Wv2'[h].T; bv2'[h]],
    # col 65h+64 = [0...0, 1] (ones column -> softmax denominator row)
    wv2p, bv2p = a["Wv2"][perm], a["bv2"][perm]
    rvm = np.zeros((D8 + 1, H * (DIM + 1)), f)
    for h in range(H):
        c0 = h * (DIM + 1)
        rvm[0:D8, c0:c0 + DIM] = wv2p[h * DIM:(h + 1) * DIM].T
        rvm[D8, c0:c0 + DIM] = bv2p[h * DIM:(h + 1) * DIM]
        rvm[D8, c0 + DIM] = 1.0
    out["rv"] = rvm
    # wm1t: [64, 4, D8]: [d, h, :] = Wm1'[:, h*64+d]
    wm1p = a["Wm1"][:, perm]
    out["wm1t"] = np.ascontiguousarray(wm1p.T.reshape(H, DIM, D8).swapaxes(0, 1))
    # mlp conv1 split into x-part and msg-part
    out["wp1xt"] = np.ascontiguousarray(
        a["Wp1"][:, 0:D].T.reshape(2, 128, TD8).swapaxes(0, 1))
    out["wp1mt"] = np.ascontiguousarray(
        a["Wp1"][:, D:TD].T.reshape(2, 128, TD8).swapaxes(0, 1))
    return {k: np.ascontiguousarray(v) for k, v in out.items()}


_NC_CACHE = None


def _get_nc():
    global _NC_CACHE
    if _NC_CACHE is None:
        _NC_CACHE = build_program()
    return _NC_CACHE


def make_in_maps(inputs):
    w = prep_weights(inputs)
    x = np.ascontiguousarray(np.asarray(inputs["x"], np.float32))
    src = np.ascontiguousarray(np.asarray(inputs["source"], np.float32))
    in_maps = []
    for c in range(NCORES):
        b, ns = c // 4, (c % 4) * NCHUNK
        m = dict(w)
        m["x_chunk"] = np.ascontiguousarray(
            x[b].reshape(2, 128, N)[:, :, ns:ns + NCHUNK])
        m["source_b"] = np.ascontiguousarray(src[b].reshape(2, 128, M))
        in_maps.append(m)
    return in_maps


def assemble_out(results):
    out = np.empty((B, D, N), np.float32)
    for c in range(NCORES):
        b, ns = c // 4, (c % 4) * NCHUNK
        out[b].reshape(2, 128, N)[:, :, ns:ns + NCHUNK] = (
            results[c]["out_chunk"])
    return out


def kernel(**inputs):
    nc = _get_nc()
    res = bass_utils.run_bass_kernel_spmd(
        nc, make_in_maps(inputs), core_ids=list(range(NCORES)))
    return assemble_out(res.results)


# revision 8
# speedup vs baseline: 1.1506x; 1.1506x over previous
"""AttentionalPropagation (SuperGlue-style GNN message passing) on 8 trn2 cores.

Problem (hardcoded): B=2, D=256, N=M=4096, H=4 heads, head dim 64.
  q = P_q(x); k = P_k(source); v = P_v(source)      (bottleneck 1x1 convs D->D/8->D)
  msg = attn(q, k, v); merged = P_m(msg)            (per-head softmax over M)
  out = Conv(relu(BN(Conv(cat[x, merged]))))        (512->64->256)

Sharding: 8 cores = (batch b in {0,1}) x (query chunk of 1024).  Each core
computes k/v for its full batch row (cheap, duplicated 4x) and attention +
MLP for its 1024 query columns.  Weights replicated.  No collectives.

Layout trick: everything stays channels-on-partitions except attention scores,
which are computed transposed (keys m on partitions, queries n free) so that
softmax normalization folds into the PE: the value matrix vT carries an extra
all-ones column per head, making row 64 of the msg-PSUM the softmax
denominator.  Head channels are made contiguous by permuting weight rows/cols
on the host.  Big matmuls run as float32r (full PE rate; fp32 is 4x slower).
"""

import numpy as np

import concourse.bass as bass
import concourse.mybir as mybir
import concourse.tile as tile
from concourse import bacc, bass_utils

B, D, N, M, H = 2, 256, 4096, 4096, 4
DIM = D // H       # 64
D8 = D // 8        # 32
TD = 2 * D         # 512
TD8 = TD // 8      # 64
BN_EPS = 1e-5
NCORES = 8
NCHUNK = N // 4    # query columns per core
NT = 512           # n tile (fp32 matmul moving-operand / PSUM bank limit)
NTILES = NCHUNK // NT          # 2
MT = 512           # source m tile for k/v projection stage
MTILES = M // MT               # 8
MC = 128           # m chunk (scores PSUM partition dim)
MCHUNKS = M // MC              # 32
F32 = mybir.dt.float32
F32R = mybir.dt.float32r
BF16 = mybir.dt.bfloat16
AF = mybir.ActivationFunctionType


def _mm(nc, out, lhsT, rhs, start, stop):
    nc.tensor.matmul(out, lhsT, rhs, start=start, stop=stop)


def build_body(ctx, tc: tile.TileContext, io):
    nc = tc.nc
    x_d = io["x_chunk"]          # [2, 128, NCHUNK]  (channel-chunk, partition, n)
    src_d = io["source_b"]       # [2, 128, M]
    out_d = io["out_chunk"]      # [2, 128, NCHUNK]

    consts = ctx.enter_context(tc.tile_pool(name="consts", bufs=1))
    big = ctx.enter_context(tc.tile_pool(name="big", bufs=1))
    srcp = ctx.enter_context(tc.tile_pool(name="srcp", bufs=3))
    kv1p = ctx.enter_context(tc.tile_pool(name="kv1p", bufs=3))
    ep = ctx.enter_context(tc.tile_pool(name="ep", bufs=6))
    nrm = ctx.enter_context(tc.tile_pool(name="nrm", bufs=4))
    pp = ctx.enter_context(tc.tile_pool(name="pp", bufs=2, space="PSUM"))
    pps = ctx.enter_context(tc.tile_pool(name="pps", bufs=4, space="PSUM"))
    ppm = ctx.enter_context(tc.tile_pool(name="ppm", bufs=2, space="PSUM"))

    # ---- constants / weights ----
    def wtile(name, shape, dt=F32R):
        t = consts.tile(shape, dt, name=name)
        nc.sync.dma_start(out=t, in_=io[name])
        return t

    wq1t = wtile("wq1t", [128, 2, D8])  # f32r: x path
    wk1t = wtile("wk1t", [128, 2, D8], BF16)
    wv1t = wtile("wv1t", [128, 2, D8], BF16)
    bq1 = wtile("bq1", [D8, 1], F32)
    bk1 = wtile("bk1", [D8, 1], F32)
    bv1 = wtile("bv1", [D8, 1], F32)
    wq2t = wtile("wq2t", [D8 + 1, 2, 128], BF16)
    wk2t = wtile("wk2t", [D8 + 1, 2, 128], BF16)
    rv = wtile("rv", [D8 + 1, H * (DIM + 1)], BF16)
    wm1t = wtile("wm1t", [DIM, H, D8], BF16)
    bm1 = wtile("bm1", [1, D8])
    wm2t = wtile("wm2t", [D8 + 1, 2, 128], BF16)
    wp1xt = wtile("wp1xt", [128, 2, TD8])
    wp1mt = wtile("wp1mt", [128, 2, TD8], BF16)
    bp1 = wtile("bp1", [1, TD8])
    g1s = wtile("g1s", [TD8, 1], F32)
    be1 = wtile("be1", [TD8, 1], F32)
    wp2t = wtile("wp2t", [TD8 + 1, 2, 128])

    ones_row = wtile("ones", [1, NCHUNK])

    # ---- persistent activations ----
    x_sb = big.tile([128, 2, NCHUNK], F32R)
    for ct in range(2):
        nc.sync.dma_start(out=x_sb[:, ct, :], in_=x_d[ct])
    k_sb = big.tile([128, 2, M], BF16)            # k', heads contiguous
    vT_sb = big.tile([128, MCHUNKS, H * (DIM + 1)], BF16)
    q_sb = big.tile([128, 2, NCHUNK], BF16)
    msg_sb = big.tile([DIM, H, NCHUNK], BF16)

    # ---- k / v projections (full M, streamed in m tiles) ----
    for mt in range(MTILES):
        ms = mt * MT
        src = srcp.tile([128, 2, MT], BF16, tag="src")
        for ct in range(2):
            nc.sync.dma_start(out=src[:, ct, :], in_=src_d[ct, :, ms:ms + MT])
        k1 = kv1p.tile([D8 + 1, MT], BF16, tag="k1")
        v1 = kv1p.tile([D8 + 1, MT], BF16, tag="v1")
        for (w1, b1, t1) in ((wk1t, bk1, k1), (wv1t, bv1, v1)):
            ps1 = pp.tile([D8, MT], F32, tag="pp", name="ps1")
            _mm(nc, ps1, w1[:, 0, :], src[:, 0, :], True, False)
            _mm(nc, ps1, w1[:, 1, :], src[:, 1, :], False, True)
            nc.vector.tensor_scalar_add(out=t1[0:D8, :], in0=ps1, scalar1=b1)
            nc.vector.tensor_copy(out=t1[D8:D8 + 1, :], in_=ones_row[:, 0:MT])
        for ct in range(2):
            psk = pp.tile([128, MT], F32, tag="pp", name="psk")
            _mm(nc, psk, wk2t[:, ct, :], k1, True, True)
            nc.vector.tensor_copy(out=k_sb[:, ct, ms:ms + MT], in_=psk)
        for j in range(MT // MC):
            mc = (ms // MC) + j
            psv = pp.tile([128, H * (DIM + 1)], F32, tag="pp", name="psv")
            _mm(nc, psv, v1[:, j * MC:(j + 1) * MC], rv, True, True)
            nc.vector.tensor_copy(out=vT_sb[:, mc, :], in_=psv)

    # ---- q projection (this core's n chunk) ----
    q1 = big.tile([D8 + 1, NCHUNK], BF16)
    for nt in range(NTILES):
        ns = nt * NT
        psq = pp.tile([D8, NT], F32, tag="pp", name="psq")
        _mm(nc, psq, wq1t[:, 0, :], x_sb[:, 0, ns:ns + NT], True, False)
        _mm(nc, psq, wq1t[:, 1, :], x_sb[:, 1, ns:ns + NT], False, True)
        nc.vector.tensor_scalar_add(out=q1[0:D8, ns:ns + NT], in0=psq, scalar1=bq1)
    nc.vector.tensor_copy(out=q1[D8:D8 + 1, :], in_=ones_row)
    for ct in range(2):
        for nt in range(NTILES):
            ns = nt * NT
            psq2 = pp.tile([128, NT], F32, tag="pp", name="psq2")
            _mm(nc, psq2, wq2t[:, ct, :], q1[:, ns:ns + NT], True, True)
            nc.vector.tensor_copy(out=q_sb[:, ct, ns:ns + NT], in_=psq2)

    # ---- attention ----
    # scores^T chunk: [m=128, n=NT] = k_h[:, mchunk].T @ q_h[:, ntile]
    # exp via ACT (scale folds 1/sqrt(DIM)); msg psum accumulates vT' @ exp
    # where vT' col 64 per head is all-ones -> psum row 64 = softmax denom.
    for nt in range(NTILES):
        ns = nt * NT
        for pair in range(2):          # heads (2*pair, 2*pair+1) row-packed on PE
            pm0 = ppm.tile([DIM + 1, NT], F32, tag="pm", name="pm0")
            pm1 = ppm.tile([DIM + 1, NT], F32, tag="pm", name="pm1")
            for mc in range(MCHUNKS):
                ps0 = pps.tile([128, NT], F32, tag="ps", name="ps0")
                ps1 = pps.tile([128, NT], F32, tag="ps", name="ps1")
                _mm(nc, ps0, k_sb[0:64, pair, mc * MC:(mc + 1) * MC],
                    q_sb[0:64, pair, ns:ns + NT], True, True)
                _mm(nc, ps1, k_sb[64:128, pair, mc * MC:(mc + 1) * MC],
                    q_sb[64:128, pair, ns:ns + NT], True, True)
                e0 = ep.tile([128, NT], BF16, tag="e", name="e0")
                e1 = ep.tile([128, NT], BF16, tag="e", name="e1")
                nc.scalar.activation(out=e0, in_=ps0, func=AF.Exp, scale=0.125)
                nc.scalar.activation(out=e1, in_=ps1, func=AF.Exp, scale=0.125)
                st, sp = mc == 0, mc == MCHUNKS - 1
                h0, h1 = 2 * pair, 2 * pair + 1
                _mm(nc, pm0, vT_sb[:, mc, h0 * 65:h0 * 65 + 65], e0, st, sp)
                _mm(nc, pm1, vT_sb[:, mc, h1 * 65:h1 * 65 + 65], e1, st, sp)
            for hh, pm in ((2 * pair, pm0), (2 * pair + 1, pm1)):
                rec = nrm.tile([1, NT], F32, tag="rec", name="rec")
                nc.vector.reciprocal(out=rec, in_=pm[DIM:DIM + 1, :])
                bc = nrm.tile([DIM, NT], F32, tag="bc", name="bc")
                nc.gpsimd.partition_broadcast(bc, rec)
                nc.vector.tensor_mul(out=msg_sb[:, hh, ns:ns + NT],
                                     in0=pm[0:DIM, :], in1=bc)

    # ---- merge projection ----
    m1 = big.tile([D8 + 1, NCHUNK], BF16)
    for nt in range(NTILES):
        ns = nt * NT
        psm = pp.tile([D8, NT], F32, tag="pp", name="psm")
        for h in range(H):
            _mm(nc, psm, wm1t[:, h, :], msg_sb[:, h, ns:ns + NT], h == 0, False)
        _mm(nc, psm, bm1, ones_row[:, 0:NT], False, True)
        nc.vector.tensor_copy(out=m1[0:D8, ns:ns + NT], in_=psm)
    nc.vector.tensor_copy(out=m1[D8:D8 + 1, :], in_=ones_row)
    mm_sb = big.tile([128, 2, NCHUNK], BF16)      # merged msg, unpermuted chans
    for ct in range(2):
        for nt in range(NTILES):
            ns = nt * NT
            psm2 = pp.tile([128, NT], F32, tag="pp", name="psm2")
            _mm(nc, psm2, wm2t[:, ct, :], m1[:, ns:ns + NT], True, True)
            nc.vector.tensor_copy(out=mm_sb[:, ct, ns:ns + NT], in_=psm2)

    # ---- MLP: conv(cat[x, msg]) -> BN(eval) -> relu -> conv ----
    h1 = big.tile([TD8 + 1, NCHUNK], F32R)
    for nt in range(NTILES):
        ns = nt * NT
        psh = pp.tile([TD8, NT], F32, tag="pp", name="psh")
        _mm(nc, psh, wp1xt[:, 0, :], x_sb[:, 0, ns:ns + NT], True, False)
        _mm(nc, psh, wp1xt[:, 1, :], x_sb[:, 1, ns:ns + NT], False, False)
        _mm(nc, psh, wp1mt[:, 0, :], mm_sb[:, 0, ns:ns + NT], False, False)
        _mm(nc, psh, wp1mt[:, 1, :], mm_sb[:, 1, ns:ns + NT], False, False)
        _mm(nc, psh, bp1, ones_row[:, 0:NT], False, True)
        nc.scalar.activation(out=h1[0:TD8, ns:ns + NT], in_=psh, func=AF.Relu,
                             bias=be1, scale=g1s)
    nc.vector.tensor_copy(out=h1[TD8:TD8 + 1, :], in_=ones_row)
    out_sb = big.tile([128, 2, NCHUNK], F32)
    for ct in range(2):
        for nt in range(NTILES):
            ns = nt * NT
            pso = pp.tile([128, NT], F32, tag="pp", name="pso")
            _mm(nc, pso, wp2t[:, ct, :], h1[:, ns:ns + NT], True, True)
            nc.vector.tensor_copy(out=out_sb[:, ct, ns:ns + NT], in_=pso)
        nc.sync.dma_start(out=out_d[ct], in_=out_sb[:, ct, :])


def build_program():
    nc = bacc.Bacc("TRN2", target_bir_lowering=False, debug=False)
    io = {}
    def inp(name, shape, dt=F32R):
        io[name] = nc.dram_tensor(name, shape, dt, kind="ExternalInput").ap()
    inp("x_chunk", [2, 128, NCHUNK])
    inp("source_b", [2, 128, M], BF16)
    inp("wq1t", [128, 2, D8]); inp("bq1", [D8, 1], F32)
    inp("wk1t", [128, 2, D8], BF16); inp("bk1", [D8, 1], F32)
    inp("wv1t", [128, 2, D8], BF16); inp("bv1", [D8, 1], F32)
    inp("wq2t", [D8 + 1, 2, 128], BF16)
    inp("wk2t", [D8 + 1, 2, 128], BF16)
    inp("rv", [D8 + 1, H * (DIM + 1)], BF16)
    inp("wm1t", [DIM, H, D8], BF16); inp("bm1", [1, D8])
    inp("wm2t", [D8 + 1, 2, 128], BF16)
    inp("wp1xt", [128, 2, TD8]); inp("wp1mt", [128, 2, TD8], BF16); inp("bp1", [1, TD8])
    inp("g1s", [TD8, 1], F32); inp("be1", [TD8, 1], F32)
    inp("wp2t", [TD8 + 1, 2, 128])
    inp("ones", [1, NCHUNK])
    io["out_chunk"] = nc.dram_tensor(
        "out_chunk", [2, 128, NCHUNK], F32, kind="ExternalOutput").ap()
    from contextlib import ExitStack
    with tile.TileContext(nc) as tc, ExitStack() as ctx:
        build_body(ctx, tc, io)
    nc.compile()
    return nc


def prep_weights(i):
    kernel_nchunk = NCHUNK
    import ml_dtypes
    bf = ml_dtypes.bfloat16
    """Host-side preprocessing: transposes, head-channel permutation, bias
    folding (extra contraction rows), BN folding."""
    f = np.float32
    a = {k: np.asarray(v, dtype=f) for k, v in i.items()}
    # permutation making head channels contiguous: c' = h*64+d  <- c = 4*d+h
    perm = (np.arange(H)[:, None] + H * np.arange(DIM)[None, :]).reshape(-1)

    def w1t(w):       # [D8, D] -> [128, 2, D8]
        return np.ascontiguousarray(w.T.reshape(2, 128, D8).swapaxes(0, 1))

    def w2t(w, b):    # [D, D8] x [D] -> [D8+1, 2, 128], rows = [w.T; b]
        top = w.T.reshape(D8, 2, 128)
        return np.ascontiguousarray(np.concatenate([top, b.reshape(1, 2, 128)], 0))

    out = {
        "wq1t": w1t(a["Wq1"]), "bq1": a["bq1"].reshape(D8, 1),
        "wk1t": w1t(a["Wk1"]), "bk1": a["bk1"].reshape(D8, 1),
        "wv1t": w1t(a["Wv1"]), "bv1": a["bv1"].reshape(D8, 1),
        "wq2t": w2t(a["Wq2"][perm], a["bq2"][perm]),
        "wk2t": w2t(a["Wk2"][perm], a["bk2"][perm]),
        "wm2t": w2t(a["Wm2"], a["bm2"]),
        "wp2t": np.ascontiguousarray(np.concatenate(
            [a["Wp2"].T.reshape(TD8, 2, 128), a["bp2"].reshape(1, 2, 128)], 0)),
        "bm1": a["bm1"].reshape(1, D8),
        "bp1": a["bp1"].reshape(1, TD8),
        "g1s": (a["g1"] / np.sqrt(f(1.0) + f(BN_EPS))).reshape(TD8, 1).astype(f),
        "be1": a["be1"].reshape(TD8, 1),
        "ones": np.ones((1, kernel_nchunk), f),
    }
    # rv: [33, 260]; per head h: cols [65h:65h+64] = [Wv2'[h].T; bv2'[h]],
    # col 65h+64 = [0...0, 1] (ones column -> softmax denominator row)
    wv2p, bv2p = a["Wv2"][perm], a["bv2"][perm]
    rvm = np.zeros((D8 + 1, H * (DIM + 1)), f)
    for h in range(H):
        c0 = h * (DIM + 1)
        rvm[0:D8, c0:c0 + DIM] = wv2p[h * DIM:(h + 1) * DIM].T
        rvm[D8, c0:c0 + DIM] = bv2p[h * DIM:(h + 1) * DIM]
        rvm[D8, c0 + DIM] = 1.0
    out["rv"] = rvm
    # wm1t: [64, 4, D8]: [d, h, :] = Wm1'[:, h*64+d]
    wm1p = a["Wm1"][:, perm]
    out["wm1t"] = np.ascontiguousarray(wm1p.T.reshape(H, DIM, D8).swapaxes(0, 1))
    # mlp conv1 split into x-part and msg-part
    out["wp1xt"] = np.ascontiguousarray(
        a["Wp1"][:, 0:D].T.reshape(2, 128, TD8).swapaxes(0, 1))
    out["wp1mt"] = np.ascontiguousarray(
        a["Wp1"][:, D:TD].T.reshape(2, 128, TD8).swapaxes(0, 1))
    bf16_names = {"wk1t", "wv1t", "wq2t", "wk2t", "rv", "wm1t", "wm2t", "wp1mt"}
    return {k: np.ascontiguousarray(v.astype(bf) if k in bf16_names else v)
            for k, v in out.items()}


_NC_CACHE = None


def _get_nc():
    global _NC_CACHE
    if _NC_CACHE is None:
        _NC_CACHE = build_program()
    return _NC_CACHE


def make_in_maps(inputs):
    w = prep_weights(inputs)
    x = np.ascontiguousarray(np.asarray(inputs["x"], np.float32))
    src = np.ascontiguousarray(np.asarray(inputs["source"], np.float32))
    in_maps = []
    for c in range(NCORES):
        b, ns = c // 4, (c % 4) * NCHUNK
        m = dict(w)
        m["x_chunk"] = np.ascontiguousarray(
            x[b].reshape(2, 128, N)[:, :, ns:ns + NCHUNK])
        m["source_b"] = np.ascontiguousarray(src[b].reshape(2, 128, M)).astype(
            __import__("ml_dtypes").bfloat16)
        in_maps.append(m)
    return in_maps


def assemble_out(results):
    out = np.empty((B, D, N), np.float32)
    for c in range(NCORES):
        b, ns = c // 4, (c % 4) * NCHUNK
        out[b].reshape(2, 128, N)[:, :, ns:ns + NCHUNK] = (
            results[c]["out_chunk"])
    return out


def kernel(**inputs):
    nc = _get_nc()
    res = bass_utils.run_bass_kernel_spmd(
        nc, make_in_maps(inputs), core_ids=list(range(NCORES)))
    return assemble_out(res.results)


# revision 10
# speedup vs baseline: 1.1533x; 1.0023x over previous
"""AttentionalPropagation (SuperGlue-style GNN message passing) on 8 trn2 cores.

Problem (hardcoded): B=2, D=256, N=M=4096, H=4 heads, head dim 64.
  q = P_q(x); k = P_k(source); v = P_v(source)      (bottleneck 1x1 convs D->D/8->D)
  msg = attn(q, k, v); merged = P_m(msg)            (per-head softmax over M)
  out = Conv(relu(BN(Conv(cat[x, merged]))))        (512->64->256)

Sharding: 8 cores = (batch b in {0,1}) x (query chunk of 1024).  Each core
computes k/v for its full batch row (cheap, duplicated 4x) and attention +
MLP for its 1024 query columns.  Weights replicated.  No collectives.

Layout trick: everything stays channels-on-partitions except attention scores,
which are computed transposed (keys m on partitions, queries n free) so that
softmax normalization folds into the PE: the value matrix vT carries an extra
all-ones column per head, making row 64 of the msg-PSUM the softmax
denominator.  Head channels are made contiguous by permuting weight rows/cols
on the host.  Big matmuls run as float32r (full PE rate; fp32 is 4x slower).
"""

import numpy as np

import concourse.bass as bass
import concourse.mybir as mybir
import concourse.tile as tile
from concourse import bacc, bass_utils

B, D, N, M, H = 2, 256, 4096, 4096, 4
DIM = D // H       # 64
D8 = D // 8        # 32
TD = 2 * D         # 512
TD8 = TD // 8      # 64
BN_EPS = 1e-5
NCORES = 8
NCHUNK = N // 4    # query columns per core
NT = 512           # n tile (fp32 matmul moving-operand / PSUM bank limit)
NTILES = NCHUNK // NT          # 2
MT = 512           # source m tile for k/v projection stage
MTILES = M // MT               # 8
MC = 128           # m chunk (scores PSUM partition dim)
MCHUNKS = M // MC              # 32
F32 = mybir.dt.float32
F32R = mybir.dt.float32r
BF16 = mybir.dt.bfloat16
AF = mybir.ActivationFunctionType


def _mm(nc, out, lhsT, rhs, start, stop):
    nc.tensor.matmul(out, lhsT, rhs, start=start, stop=stop)


def build_body(ctx, tc: tile.TileContext, io):
    nc = tc.nc
    x_d = io["x_chunk"]          # [2, 128, NCHUNK]  (channel-chunk, partition, n)
    src_d = io["source_b"]       # [2, 128, M]
    out_d = io["out_chunk"]      # [2, 128, NCHUNK]

    consts = ctx.enter_context(tc.tile_pool(name="consts", bufs=1))
    big = ctx.enter_context(tc.tile_pool(name="big", bufs=1))
    srcp = ctx.enter_context(tc.tile_pool(name="srcp", bufs=3))
    kv1p = ctx.enter_context(tc.tile_pool(name="kv1p", bufs=3))
    ep = ctx.enter_context(tc.tile_pool(name="ep", bufs=6))
    nrm = ctx.enter_context(tc.tile_pool(name="nrm", bufs=4))
    pp = ctx.enter_context(tc.tile_pool(name="pp", bufs=2, space="PSUM"))
    pps = ctx.enter_context(tc.tile_pool(name="pps", bufs=4, space="PSUM"))
    ppm = ctx.enter_context(tc.tile_pool(name="ppm", bufs=2, space="PSUM"))

    # ---- constants / weights ----
    def wtile(name, shape, dt=F32R):
        t = consts.tile(shape, dt, name=name)
        nc.sync.dma_start(out=t, in_=io[name])
        return t

    wq1t = wtile("wq1t", [128, 2, D8])  # f32r: x path
    wk1t = wtile("wk1t", [128, 2, D8], BF16)
    wv1t = wtile("wv1t", [128, 2, D8], BF16)
    bq1 = wtile("bq1", [D8, 1], F32)
    bk1 = wtile("bk1", [D8, 1], F32)
    bv1 = wtile("bv1", [D8, 1], F32)
    wq2t = wtile("wq2t", [D8 + 1, 2, 128], BF16)
    wk2t = wtile("wk2t", [D8 + 1, 2, 128], BF16)
    rv = wtile("rv", [D8 + 1, H * (DIM + 1)], BF16)
    wm1t = wtile("wm1t", [DIM, H, D8], BF16)
    bm1 = wtile("bm1", [1, D8])
    wm2t = wtile("wm2t", [D8 + 1, 2, 128], BF16)
    wp1xt = wtile("wp1xt", [128, 2, TD8])
    wp1mt = wtile("wp1mt", [128, 2, TD8], BF16)
    bp1 = wtile("bp1", [1, TD8])
    g1s = wtile("g1s", [TD8, 1], F32)
    be1 = wtile("be1", [TD8, 1], F32)
    wp2t = wtile("wp2t", [TD8 + 1, 2, 128])

    ones_row = wtile("ones", [1, NCHUNK])

    # ---- persistent activations ----
    x_sb = big.tile([128, 2, NCHUNK], F32R)
    for ct in range(2):
        nc.sync.dma_start(out=x_sb[:, ct, :], in_=x_d[ct])
    k_sb = big.tile([128, 2, M], BF16)            # k', heads contiguous
    vT_sb = big.tile([128, MCHUNKS, H * (DIM + 1)], BF16)
    q_sb = big.tile([128, 2, NCHUNK], BF16)
    msg_sb = big.tile([DIM, H, NCHUNK], BF16)

    # ---- k / v projections (full M, streamed in m tiles) ----
    for mt in range(MTILES):
        ms = mt * MT
        src = srcp.tile([128, 2, MT], BF16, tag="src")
        for ct in range(2):
            nc.sync.dma_start(out=src[:, ct, :], in_=src_d[ct, :, ms:ms + MT])
        k1 = kv1p.tile([D8 + 1, MT], BF16, tag="k1")
        v1 = kv1p.tile([D8 + 1, MT], BF16, tag="v1")
        for (w1, b1, t1) in ((wk1t, bk1, k1), (wv1t, bv1, v1)):
            ps1 = pp.tile([D8, MT], F32, tag="pp", name="ps1")
            _mm(nc, ps1, w1[:, 0, :], src[:, 0, :], True, False)
            _mm(nc, ps1, w1[:, 1, :], src[:, 1, :], False, True)
            nc.vector.tensor_scalar_add(out=t1[0:D8, :], in0=ps1, scalar1=b1)
            nc.vector.tensor_copy(out=t1[D8:D8 + 1, :], in_=ones_row[:, 0:MT])
        for ct in range(2):
            psk = pp.tile([128, MT], F32, tag="pp", name="psk")
            _mm(nc, psk, wk2t[:, ct, :], k1, True, True)
            nc.vector.tensor_copy(out=k_sb[:, ct, ms:ms + MT], in_=psk)
        for j in range(MT // MC):
            mc = (ms // MC) + j
            psv = pp.tile([128, H * (DIM + 1)], F32, tag="pp", name="psv")
            _mm(nc, psv, v1[:, j * MC:(j + 1) * MC], rv, True, True)
            nc.vector.tensor_copy(out=vT_sb[:, mc, :], in_=psv)

    # ---- q projection (this core's n chunk) ----
    q1 = big.tile([D8 + 1, NCHUNK], BF16)
    for nt in range(NTILES):
        ns = nt * NT
        psq = pp.tile([D8, NT], F32, tag="pp", name="psq")
        _mm(nc, psq, wq1t[:, 0, :], x_sb[:, 0, ns:ns + NT], True, False)
        _mm(nc, psq, wq1t[:, 1, :], x_sb[:, 1, ns:ns + NT], False, True)
        nc.vector.tensor_scalar_add(out=q1[0:D8, ns:ns + NT], in0=psq, scalar1=bq1)
    nc.vector.tensor_copy(out=q1[D8:D8 + 1, :], in_=ones_row)
    for ct in range(2):
        for nt in range(NTILES):
            ns = nt * NT
            psq2 = pp.tile([128, NT], F32, tag="pp", name="psq2")
            _mm(nc, psq2, wq2t[:, ct, :], q1[:, ns:ns + NT], True, True)
            nc.vector.tensor_copy(out=q_sb[:, ct, ns:ns + NT], in_=psq2)

    # ---- attention ----
    # scores^T chunk: [m=128, n=NT] = k_h[:, mchunk].T @ q_h[:, ntile]
    # exp via ACT (scale folds 1/sqrt(DIM)); msg psum accumulates vT' @ exp
    # where vT' col 64 per head is all-ones -> psum row 64 = softmax denom.
    for nt in range(NTILES):
        ns = nt * NT
        for pair in range(2):          # heads (2*pair, 2*pair+1) row-packed on PE
            pm0 = ppm.tile([DIM + 1, NT], F32, tag="pm", name="pm0")
            pm1 = ppm.tile([DIM + 1, NT], F32, tag="pm", name="pm1")

            def emit_scores(mc):
                ps0 = pps.tile([128, NT], F32, tag="ps", name="ps0")
                ps1 = pps.tile([128, NT], F32, tag="ps", name="ps1")
                _mm(nc, ps0, k_sb[0:64, pair, mc * MC:(mc + 1) * MC],
                    q_sb[0:64, pair, ns:ns + NT], True, True)
                _mm(nc, ps1, k_sb[64:128, pair, mc * MC:(mc + 1) * MC],
                    q_sb[64:128, pair, ns:ns + NT], True, True)
                e0 = ep.tile([128, NT], BF16, tag="e", name="e0")
                e1 = ep.tile([128, NT], BF16, tag="e", name="e1")
                nc.scalar.activation(out=e0, in_=ps0, func=AF.Exp, scale=0.125)
                nc.scalar.activation(out=e1, in_=ps1, func=AF.Exp, scale=0.125)
                return e0, e1

            # software pipeline: scores/exp for chunk mc+1 are emitted ahead
            # of the msg matmuls for chunk mc, so the PE queue never stalls
            # waiting on the ACT engine's exp.
            pend = emit_scores(0)
            for mc in range(MCHUNKS):
                nxt = emit_scores(mc + 1) if mc + 1 < MCHUNKS else None
                e0, e1 = pend
                st, sp = mc == 0, mc == MCHUNKS - 1
                h0, h1 = 2 * pair, 2 * pair + 1
                _mm(nc, pm0, vT_sb[:, mc, h0 * 65:h0 * 65 + 65], e0, st, sp)
                _mm(nc, pm1, vT_sb[:, mc, h1 * 65:h1 * 65 + 65], e1, st, sp)
                pend = nxt
            for hh, pm in ((2 * pair, pm0), (2 * pair + 1, pm1)):
                rec = nrm.tile([1, NT], F32, tag="rec", name="rec")
                nc.vector.reciprocal(out=rec, in_=pm[DIM:DIM + 1, :])
                bc = nrm.tile([DIM, NT], F32, tag="bc", name="bc")
                nc.gpsimd.partition_broadcast(bc, rec)
                nc.vector.tensor_mul(out=msg_sb[:, hh, ns:ns + NT],
                                     in0=pm[0:DIM, :], in1=bc)

    # ---- merge projection ----
    m1 = big.tile([D8 + 1, NCHUNK], BF16)
    for nt in range(NTILES):
        ns = nt * NT
        psm = pp.tile([D8, NT], F32, tag="pp", name="psm")
        for h in range(H):
            _mm(nc, psm, wm1t[:, h, :], msg_sb[:, h, ns:ns + NT], h == 0, False)
        _mm(nc, psm, bm1, ones_row[:, 0:NT], False, True)
        nc.vector.tensor_copy(out=m1[0:D8, ns:ns + NT], in_=psm)
    nc.vector.tensor_copy(out=m1[D8:D8 + 1, :], in_=ones_row)
    mm_sb = big.tile([128, 2, NCHUNK], BF16)      # merged msg, unpermuted chans
    for ct in range(2):
        for nt in range(NTILES):
            ns = nt * NT
            psm2 = pp.tile([128, NT], F32, tag="pp", name="psm2")
            _mm(nc, psm2, wm2t[:, ct, :], m1[:, ns:ns + NT], True, True)
            nc.vector.tensor_copy(out=mm_sb[:, ct, ns:ns + NT], in_=psm2)

    # ---- MLP: conv(cat[x, msg]) -> BN(eval) -> relu -> conv ----
    h1 = big.tile([TD8 + 1, NCHUNK], F32R)
    for nt in range(NTILES):
        ns = nt * NT
        psh = pp.tile([TD8, NT], F32, tag="pp", name="psh")
        _mm(nc, psh, wp1xt[:, 0, :], x_sb[:, 0, ns:ns + NT], True, False)
        _mm(nc, psh, wp1xt[:, 1, :], x_sb[:, 1, ns:ns + NT], False, False)
        _mm(nc, psh, wp1mt[:, 0, :], mm_sb[:, 0, ns:ns + NT], False, False)
        _mm(nc, psh, wp1mt[:, 1, :], mm_sb[:, 1, ns:ns + NT], False, False)
        _mm(nc, psh, bp1, ones_row[:, 0:NT], False, True)
        nc.scalar.activation(out=h1[0:TD8, ns:ns + NT], in_=psh, func=AF.Relu,
                             bias=be1, scale=g1s)
    nc.vector.tensor_copy(out=h1[TD8:TD8 + 1, :], in_=ones_row)
    out_sb = big.tile([128, 2, NCHUNK], F32)
    for ct in range(2):
        for nt in range(NTILES):
            ns = nt * NT
            pso = pp.tile([128, NT], F32, tag="pp", name="pso")
            _mm(nc, pso, wp2t[:, ct, :], h1[:, ns:ns + NT], True, True)
            nc.vector.tensor_copy(out=out_sb[:, ct, ns:ns + NT], in_=pso)
        nc.sync.dma_start(out=out_d[ct], in_=out_sb[:, ct, :])


def build_program():
    nc = bacc.Bacc("TRN2", target_bir_lowering=False, debug=False)
    io = {}
    def inp(name, shape, dt=F32R):
        io[name] = nc.dram_tensor(name, shape, dt, kind="ExternalInput").ap()
    inp("x_chunk", [2, 128, NCHUNK])
    inp("source_b", [2, 128, M], BF16)
    inp("wq1t", [128, 2, D8]); inp("bq1", [D8, 1], F32)
    inp("wk1t", [128, 2, D8], BF16); inp("bk1", [D8, 1], F32)
    inp("wv1t", [128, 2, D8], BF16); inp("bv1", [D8, 1], F32)
    inp("wq2t", [D8 + 1, 2, 128], BF16)
    inp("wk2t", [D8 + 1, 2, 128], BF16)
    inp("rv", [D8 + 1, H * (DIM + 1)], BF16)
    inp("wm1t", [DIM, H, D8], BF16); inp("bm1", [1, D8])
    inp("wm2t", [D8 + 1, 2, 128], BF16)
    inp("wp1xt", [128, 2, TD8]); inp("wp1mt", [128, 2, TD8], BF16); inp("bp1", [1, TD8])
    inp("g1s", [TD8, 1], F32); inp("be1", [TD8, 1], F32)
    inp("wp2t", [TD8 + 1, 2, 128])
    inp("ones", [1, NCHUNK])
    io["out_chunk"] = nc.dram_tensor(
        "out_chunk", [2, 128, NCHUNK], F32, kind="ExternalOutput").ap()
    from contextlib import ExitStack
    with tile.TileContext(nc) as tc, ExitStack() as ctx:
        build_body(ctx, tc, io)
    nc.compile()
    return nc


def prep_weights(i):
    kernel_nchunk = NCHUNK
    import ml_dtypes
    bf = ml_dtypes.bfloat16
    """Host-side preprocessing: transposes, head-channel permutation, bias
    folding (extra contraction rows), BN folding."""
    f = np.float32
    a = {k: np.asarray(v, dtype=f) for k, v in i.items()}
    # permutation making head channels contiguous: c' = h*64+d  <- c = 4*d+h
    perm = (np.arange(H)[:, None] + H * np.arange(DIM)[None, :]).reshape(-1)

    def w1t(w):       # [D8, D] -> [128, 2, D8]
        return np.ascontiguousarray(w.T.reshape(2, 128, D8).swapaxes(0, 1))

    def w2t(w, b):    # [D, D8] x [D] -> [D8+1, 2, 128], rows = [w.T; b]
        top = w.T.reshape(D8, 2, 128)
        return np.ascontiguousarray(np.concatenate([top, b.reshape(1, 2, 128)], 0))

    out = {
        "wq1t": w1t(a["Wq1"]), "bq1": a["bq1"].reshape(D8, 1),
        "wk1t": w1t(a["Wk1"]), "bk1": a["bk1"].reshape(D8, 1),
        "wv1t": w1t(a["Wv1"]), "bv1": a["bv1"].reshape(D8, 1),
        "wq2t": w2t(a["Wq2"][perm], a["bq2"][perm]),
        "wk2t": w2t(a["Wk2"][perm], a["bk2"][perm]),
        "wm2t": w2t(a["Wm2"], a["bm2"]),
        "wp2t": np.ascontiguousarray(np.concatenate(
            [a["Wp2"].T.reshape(TD8, 2, 128), a["bp2"].reshape(1, 2, 128)], 0)),
        "bm1": a["bm1"].reshape(1, D8),
        "bp1": a["bp1"].reshape(1, TD8),
        "g1s": (a["g1"] / np.sqrt(f(1.0) + f(BN_EPS))).reshape(TD8, 1).astype(f),
        "be1": a["be1"].reshape(TD8, 1),
        "ones": np.ones((1, kernel_nchunk), f),
    }
    # rv: [33, 260]; per head h: cols [65h:65h+64] = [Wv2'[h].T; bv2'[h]],
    # col 65h+64 = [0...0, 1] (ones column -> softmax denominator row)
    wv2p, bv2p = a["Wv2"][perm], a["bv2"][perm]
    rvm = np.zeros((D8 + 1, H * (DIM + 1)), f)
    for h in range(H):
        c0 = h * (DIM + 1)
        rvm[0:D8, c0:c0 + DIM] = wv2p[h * DIM:(h + 1) * DIM].T
        rvm[D8, c0:c0 + DIM] = bv2p[h * DIM:(h + 1) * DIM]
        rvm[D8, c0 + DIM] = 1.0
    out["rv"] = rvm
    # wm1t: [64, 4, D8]: [d, h, :] = Wm1'[:, h*64+d]
    wm1p = a["Wm1"][:, perm]
    out["wm1t"] = np.ascontiguousarray(wm1p.T.reshape(H, DIM, D8).swapaxes(0, 1))
    # mlp conv1 split into x-part and msg-part
    out["wp1xt"] = np.ascontiguousarray(
        a["Wp1"][:, 0:D].T.reshape(2, 128, TD8).swapaxes(0, 1))
    out["wp1mt"] = np.ascontiguousarray(
        a["Wp1"][:, D:TD].T.reshape(2, 128, TD8).swapaxes(0, 1))
    bf16_names = {"wk1t", "wv1t", "wq2t", "wk2t", "rv", "wm1t", "wm2t", "wp1mt"}
    return {k: np.ascontiguousarray(v.astype(bf) if k in bf16_names else v)
            for k, v in out.items()}


_NC_CACHE = None


def _get_nc():
    global _NC_CACHE
    if _NC_CACHE is None:
        _NC_CACHE = build_program()
    return _NC_CACHE


def make_in_maps(inputs):
    w = prep_weights(inputs)
    x = np.ascontiguousarray(np.asarray(inputs["x"], np.float32))
    src = np.ascontiguousarray(np.asarray(inputs["source"], np.float32))
    in_maps = []
    for c in range(NCORES):
        b, ns = c // 4, (c % 4) * NCHUNK
        m = dict(w)
        m["x_chunk"] = np.ascontiguousarray(
            x[b].reshape(2, 128, N)[:, :, ns:ns + NCHUNK])
        m["source_b"] = np.ascontiguousarray(src[b].reshape(2, 128, M)).astype(
            __import__("ml_dtypes").bfloat16)
        in_maps.append(m)
    return in_maps


def assemble_out(results):
    out = np.empty((B, D, N), np.float32)
    for c in range(NCORES):
        b, ns = c // 4, (c % 4) * NCHUNK
        out[b].reshape(2, 128, N)[:, :, ns:ns + NCHUNK] = (
            results[c]["out_chunk"])
    return out


def kernel(**inputs):
    nc = _get_nc()
    res = bass_utils.run_bass_kernel_spmd(
        nc, make_in_maps(inputs), core_ids=list(range(NCORES)))
    return assemble_out(res.results)


# revision 12
# speedup vs baseline: 1.2919x; 1.1202x over previous
"""AttentionalPropagation (SuperGlue-style GNN message passing) on 8 trn2 cores.

Problem (hardcoded): B=2, D=256, N=M=4096, H=4 heads, head dim 64.
  q = P_q(x); k = P_k(source); v = P_v(source)      (bottleneck 1x1 convs D->D/8->D)
  msg = attn(q, k, v); merged = P_m(msg)            (per-head softmax over M)
  out = Conv(relu(BN(Conv(cat[x, merged]))))        (512->64->256)

Sharding: 8 cores = (batch b in {0,1}) x (query chunk of 1024).  Each core
computes k/v for its full batch row (cheap, duplicated 4x) and attention +
MLP for its 1024 query columns.  Weights replicated.  No collectives.

Layout trick: everything stays channels-on-partitions except attention scores,
which are computed transposed (keys m on partitions, queries n free) so that
softmax normalization folds into the PE: the value matrix vT carries an extra
all-ones column per head, making row 64 of the msg-PSUM the softmax
denominator.  Head channels are made contiguous by permuting weight rows/cols
on the host.  Big matmuls run as float32r (full PE rate; fp32 is 4x slower).
"""

import numpy as np

import concourse.bass as bass
import concourse.mybir as mybir
import concourse.tile as tile
from concourse import bacc, bass_utils

B, D, N, M, H = 2, 256, 4096, 4096, 4
DIM = D // H       # 64
D8 = D // 8        # 32
TD = 2 * D         # 512
TD8 = TD // 8      # 64
BN_EPS = 1e-5
NCORES = 8
NCHUNK = N // 4    # query columns per core
NT = 512           # n tile (fp32 matmul moving-operand / PSUM bank limit)
NTILES = NCHUNK // NT          # 2
MT = 512           # source m tile for k/v projection stage
MTILES = M // MT               # 8
MC = 128           # m chunk (scores PSUM partition dim)
MCHUNKS = M // MC              # 32
F32 = mybir.dt.float32
F32R = mybir.dt.float32r
BF16 = mybir.dt.bfloat16
AF = mybir.ActivationFunctionType


def _mm(nc, out, lhsT, rhs, start, stop):
    nc.tensor.matmul(out, lhsT, rhs, start=start, stop=stop)


def build_body(ctx, tc: tile.TileContext, io):
    nc = tc.nc
    x_d = io["x_chunk"]          # [2, 128, NCHUNK]  (channel-chunk, partition, n)
    src_d = io["source_b"]       # [2, 128, M]
    out_d = io["out_chunk"]      # [2, 128, NCHUNK]

    consts = ctx.enter_context(tc.tile_pool(name="consts", bufs=1))
    big = ctx.enter_context(tc.tile_pool(name="big", bufs=1))
    srcp = ctx.enter_context(tc.tile_pool(name="srcp", bufs=3))
    kv1p = ctx.enter_context(tc.tile_pool(name="kv1p", bufs=3))
    ep = ctx.enter_context(tc.tile_pool(name="ep", bufs=6))
    nrm = ctx.enter_context(tc.tile_pool(name="nrm", bufs=4))
    pp = ctx.enter_context(tc.tile_pool(name="pp", bufs=2, space="PSUM"))
    pps = ctx.enter_context(tc.tile_pool(name="pps", bufs=2, space="PSUM"))
    ppm = ctx.enter_context(tc.tile_pool(name="ppm", bufs=2, space="PSUM"))

    # ---- constants / weights ----
    def wtile(name, shape, dt=F32R):
        t = consts.tile(shape, dt, name=name)
        nc.sync.dma_start(out=t, in_=io[name])
        return t

    wq1t = wtile("wq1t", [128, 2, D8])  # f32r: x path
    wk1t = wtile("wk1t", [128, 2, D8], BF16)
    wv1t = wtile("wv1t", [128, 2, D8], BF16)
    bq1 = wtile("bq1", [D8, 1], F32)
    bk1 = wtile("bk1", [D8, 1], F32)
    bv1 = wtile("bv1", [D8, 1], F32)
    wq2t = wtile("wq2t", [D8 + 1, 2, 128], BF16)
    wk2t = wtile("wk2t", [D8 + 1, 2, 128], BF16)
    rv = wtile("rv", [D8 + 1, H * (DIM + 1)], BF16)
    wm1t = wtile("wm1t", [DIM, H, D8], BF16)
    bm1 = wtile("bm1", [1, D8])
    wm2t = wtile("wm2t", [D8 + 1, 2, 128], BF16)
    wp1xt = wtile("wp1xt", [128, 2, TD8])
    wp1mt = wtile("wp1mt", [128, 2, TD8], BF16)
    bp1 = wtile("bp1", [1, TD8])
    g1s = wtile("g1s", [TD8, 1], F32)
    be1 = wtile("be1", [TD8, 1], F32)
    wp2t = wtile("wp2t", [TD8 + 1, 2, 128])

    ones_row = wtile("ones", [1, NCHUNK])

    # ---- persistent activations ----
    x_sb = big.tile([128, 2, NCHUNK], F32R)
    for ct in range(2):
        nc.sync.dma_start(out=x_sb[:, ct, :], in_=x_d[ct])
    k_sb = big.tile([128, 2, M], BF16)            # k', heads contiguous
    vT_sb = big.tile([128, MCHUNKS, H * (DIM + 1)], BF16)
    q_sb = big.tile([128, 2, NCHUNK], BF16)
    msg_sb = big.tile([DIM, H, NCHUNK], BF16)

    # ---- k / v projections (full M, streamed in m tiles) ----
    for mt in range(MTILES):
        ms = mt * MT
        src = srcp.tile([128, 2, MT], BF16, tag="src")
        for ct in range(2):
            nc.sync.dma_start(out=src[:, ct, :], in_=src_d[ct, :, ms:ms + MT])
        k1 = kv1p.tile([D8 + 1, MT], BF16, tag="k1")
        v1 = kv1p.tile([D8 + 1, MT], BF16, tag="v1")
        for (w1, b1, t1) in ((wk1t, bk1, k1), (wv1t, bv1, v1)):
            ps1 = pp.tile([D8, MT], F32, tag="pp", name="ps1")
            _mm(nc, ps1, w1[:, 0, :], src[:, 0, :], True, False)
            _mm(nc, ps1, w1[:, 1, :], src[:, 1, :], False, True)
            nc.vector.tensor_scalar_add(out=t1[0:D8, :], in0=ps1, scalar1=b1)
            nc.vector.tensor_copy(out=t1[D8:D8 + 1, :], in_=ones_row[:, 0:MT])
        for ct in range(2):
            psk = pp.tile([128, MT], F32, tag="pp", name="psk")
            _mm(nc, psk, wk2t[:, ct, :], k1, True, True)
            nc.vector.tensor_copy(out=k_sb[:, ct, ms:ms + MT], in_=psk)
        for j in range(MT // MC):
            mc = (ms // MC) + j
            psv = pp.tile([128, H * (DIM + 1)], F32, tag="pp", name="psv")
            _mm(nc, psv, v1[:, j * MC:(j + 1) * MC], rv, True, True)
            nc.vector.tensor_copy(out=vT_sb[:, mc, :], in_=psv)

    # ---- q projection (this core's n chunk) ----
    q1 = big.tile([D8 + 1, NCHUNK], BF16)
    for nt in range(NTILES):
        ns = nt * NT
        psq = pp.tile([D8, NT], F32, tag="pp", name="psq")
        _mm(nc, psq, wq1t[:, 0, :], x_sb[:, 0, ns:ns + NT], True, False)
        _mm(nc, psq, wq1t[:, 1, :], x_sb[:, 1, ns:ns + NT], False, True)
        nc.vector.tensor_scalar_add(out=q1[0:D8, ns:ns + NT], in0=psq, scalar1=bq1)
    nc.vector.tensor_copy(out=q1[D8:D8 + 1, :], in_=ones_row)
    for ct in range(2):
        for nt in range(NTILES):
            ns = nt * NT
            psq2 = pp.tile([128, NT], F32, tag="pp", name="psq2")
            _mm(nc, psq2, wq2t[:, ct, :], q1[:, ns:ns + NT], True, True)
            nc.vector.tensor_copy(out=q_sb[:, ct, ns:ns + NT], in_=psq2)

    # ---- attention ----
    # scores^T chunk: [m=128, n=NT] = k_h[:, mchunk].T @ q_h[:, ntile]
    # exp via ACT (scale folds 1/sqrt(DIM)); msg psum accumulates vT' @ exp
    # where vT' col 64 per head is all-ones -> psum row 64 = softmax denom.
    # 2 chunks per exp instruction (amortizes the ~293ns ACT fixed cost);
    # software pipeline keeps the PE queue dense so HAM reaches 2.4 GHz.
    BC = 2                      # chunks per exp batch
    NBATCH = MCHUNKS // BC      # 16
    for nt in range(NTILES):
        ns = nt * NT
        for h in range(H):
            ct, half = h // 2, (h % 2) * 64
            pm = ppm.tile([DIM + 1, NT], F32, tag="pm", name="pm")

            def emit_batch(bi):
                ps = pps.tile([128, BC, NT], F32, tag="ps", name="ps")
                for j in range(BC):
                    mc = bi * BC + j
                    _mm(nc, ps[:, j, :],
                        k_sb[half:half + 64, ct, mc * MC:(mc + 1) * MC],
                        q_sb[half:half + 64, ct, ns:ns + NT], True, True)
                e = ep.tile([128, BC, NT], BF16, tag="e", name="e")
                nc.scalar.activation(out=e, in_=ps, func=AF.Exp, scale=0.125)
                return e

            pend = emit_batch(0)
            for bi in range(NBATCH):
                nxt = emit_batch(bi + 1) if bi + 1 < NBATCH else None
                for j in range(BC):
                    mc = bi * BC + j
                    _mm(nc, pm, vT_sb[:, mc, h * 65:h * 65 + 65],
                        pend[:, j, :], mc == 0, mc == MCHUNKS - 1)
                pend = nxt
            rec = nrm.tile([1, NT], F32, tag="rec", name="rec")
            nc.vector.reciprocal(out=rec, in_=pm[DIM:DIM + 1, :])
            bc = nrm.tile([DIM, NT], F32, tag="bc", name="bc")
            nc.gpsimd.partition_broadcast(bc, rec)
            nc.vector.tensor_mul(out=msg_sb[:, h, ns:ns + NT],
                                 in0=pm[0:DIM, :], in1=bc)

    # ---- merge projection ----
    m1 = big.tile([D8 + 1, NCHUNK], BF16)
    for nt in range(NTILES):
        ns = nt * NT
        psm = pp.tile([D8, NT], F32, tag="pp", name="psm")
        for h in range(H):
            _mm(nc, psm, wm1t[:, h, :], msg_sb[:, h, ns:ns + NT], h == 0, False)
        _mm(nc, psm, bm1, ones_row[:, 0:NT], False, True)
        nc.vector.tensor_copy(out=m1[0:D8, ns:ns + NT], in_=psm)
    nc.vector.tensor_copy(out=m1[D8:D8 + 1, :], in_=ones_row)
    mm_sb = big.tile([128, 2, NCHUNK], BF16)      # merged msg, unpermuted chans
    for ct in range(2):
        for nt in range(NTILES):
            ns = nt * NT
            psm2 = pp.tile([128, NT], F32, tag="pp", name="psm2")
            _mm(nc, psm2, wm2t[:, ct, :], m1[:, ns:ns + NT], True, True)
            nc.vector.tensor_copy(out=mm_sb[:, ct, ns:ns + NT], in_=psm2)

    # ---- MLP: conv(cat[x, msg]) -> BN(eval) -> relu -> conv ----
    h1 = big.tile([TD8 + 1, NCHUNK], F32R)
    for nt in range(NTILES):
        ns = nt * NT
        psh = pp.tile([TD8, NT], F32, tag="pp", name="psh")
        _mm(nc, psh, wp1xt[:, 0, :], x_sb[:, 0, ns:ns + NT], True, False)
        _mm(nc, psh, wp1xt[:, 1, :], x_sb[:, 1, ns:ns + NT], False, False)
        _mm(nc, psh, wp1mt[:, 0, :], mm_sb[:, 0, ns:ns + NT], False, False)
        _mm(nc, psh, wp1mt[:, 1, :], mm_sb[:, 1, ns:ns + NT], False, False)
        _mm(nc, psh, bp1, ones_row[:, 0:NT], False, True)
        nc.scalar.activation(out=h1[0:TD8, ns:ns + NT], in_=psh, func=AF.Relu,
                             bias=be1, scale=g1s)
    nc.vector.tensor_copy(out=h1[TD8:TD8 + 1, :], in_=ones_row)
    out_sb = big.tile([128, 2, NCHUNK], F32)
    for ct in range(2):
        for nt in range(NTILES):
            ns = nt * NT
            pso = pp.tile([128, NT], F32, tag="pp", name="pso")
            _mm(nc, pso, wp2t[:, ct, :], h1[:, ns:ns + NT], True, True)
            nc.vector.tensor_copy(out=out_sb[:, ct, ns:ns + NT], in_=pso)
        nc.sync.dma_start(out=out_d[ct], in_=out_sb[:, ct, :])


def build_program():
    nc = bacc.Bacc("TRN2", target_bir_lowering=False, debug=False)
    io = {}
    def inp(name, shape, dt=F32R):
        io[name] = nc.dram_tensor(name, shape, dt, kind="ExternalInput").ap()
    inp("x_chunk", [2, 128, NCHUNK])
    inp("source_b", [2, 128, M], BF16)
    inp("wq1t", [128, 2, D8]); inp("bq1", [D8, 1], F32)
    inp("wk1t", [128, 2, D8], BF16); inp("bk1", [D8, 1], F32)
    inp("wv1t", [128, 2, D8], BF16); inp("bv1", [D8, 1], F32)
    inp("wq2t", [D8 + 1, 2, 128], BF16)
    inp("wk2t", [D8 + 1, 2, 128], BF16)
    inp("rv", [D8 + 1, H * (DIM + 1)], BF16)
    inp("wm1t", [DIM, H, D8], BF16); inp("bm1", [1, D8])
    inp("wm2t", [D8 + 1, 2, 128], BF16)
    inp("wp1xt", [128, 2, TD8]); inp("wp1mt", [128, 2, TD8], BF16); inp("bp1", [1, TD8])
    inp("g1s", [TD8, 1], F32); inp("be1", [TD8, 1], F32)
    inp("wp2t", [TD8 + 1, 2, 128])
    inp("ones", [1, NCHUNK])
    io["out_chunk"] = nc.dram_tensor(
        "out_chunk", [2, 128, NCHUNK], F32, kind="ExternalOutput").ap()
    from contextlib import ExitStack
    with tile.TileContext(nc) as tc, ExitStack() as ctx:
        build_body(ctx, tc, io)
    nc.compile()
    return nc


def prep_weights(i):
    kernel_nchunk = NCHUNK
    import ml_dtypes
    bf = ml_dtypes.bfloat16
    """Host-side preprocessing: transposes, head-channel permutation, bias
    folding (extra contraction rows), BN folding."""
    f = np.float32
    a = {k: np.asarray(v, dtype=f) for k, v in i.items()}
    # permutation making head channels contiguous: c' = h*64+d  <- c = 4*d+h
    perm = (np.arange(H)[:, None] + H * np.arange(DIM)[None, :]).reshape(-1)

    def w1t(w):       # [D8, D] -> [128, 2, D8]
        return np.ascontiguousarray(w.T.reshape(2, 128, D8).swapaxes(0, 1))

    def w2t(w, b):    # [D, D8] x [D] -> [D8+1, 2, 128], rows = [w.T; b]
        top = w.T.reshape(D8, 2, 128)
        return np.ascontiguousarray(np.concatenate([top, b.reshape(1, 2, 128)], 0))

    out = {
        "wq1t": w1t(a["Wq1"]), "bq1": a["bq1"].reshape(D8, 1),
        "wk1t": w1t(a["Wk1"]), "bk1": a["bk1"].reshape(D8, 1),
        "wv1t": w1t(a["Wv1"]), "bv1": a["bv1"].reshape(D8, 1),
        "wq2t": w2t(a["Wq2"][perm], a["bq2"][perm]),
        "wk2t": w2t(a["Wk2"][perm], a["bk2"][perm]),
        "wm2t": w2t(a["Wm2"], a["bm2"]),
        "wp2t": np.ascontiguousarray(np.concatenate(
            [a["Wp2"].T.reshape(TD8, 2, 128), a["bp2"].reshape(1, 2, 128)], 0)),
        "bm1": a["bm1"].reshape(1, D8),
        "bp1": a["bp1"].reshape(1, TD8),
        "g1s": (a["g1"] / np.sqrt(f(1.0) + f(BN_EPS))).reshape(TD8, 1).astype(f),
        "be1": a["be1"].reshape(TD8, 1),
        "ones": np.ones((1, kernel_nchunk), f),
    }
    # rv: [33, 260]; per head h: cols [65h:65h+64] = [Wv2'[h].T; bv2'[h]],
    # col 65h+64 = [0...0, 1] (ones column -> softmax denominator row)
    wv2p, bv2p = a["Wv2"][perm], a["bv2"][perm]
    rvm = np.zeros((D8 + 1, H * (DIM + 1)), f)
    for h in range(H):
        c0 = h * (DIM + 1)
        rvm[0:D8, c0:c0 + DIM] = wv2p[h * DIM:(h + 1) * DIM].T
        rvm[D8, c0:c0 + DIM] = bv2p[h * DIM:(h + 1) * DIM]
        rvm[D8, c0 + DIM] = 1.0
    out["rv"] = rvm
    # wm1t: [64, 4, D8]: [d, h, :] = Wm1'[:, h*64+d]
    wm1p = a["Wm1"][:, perm]
    out["wm1t"] = np.ascontiguousarray(wm1p.T.reshape(H, DIM, D8).swapaxes(0, 1))
    # mlp conv1 split into x-part and msg-part
    out["wp1xt"] = np.ascontiguousarray(
        a["Wp1"][:, 0:D].T.reshape(2, 128, TD8).swapaxes(0, 1))
    out["wp1mt"] = np.ascontiguousarray(
        a["Wp1"][:, D:TD].T.reshape(2, 128, TD8).swapaxes(0, 1))
    bf16_names = {"wk1t", "wv1t", "wq2t", "wk2t", "rv", "wm1t", "wm2t", "wp1mt"}
    return {k: np.ascontiguousarray(v.astype(bf) if k in bf16_names else v)
            for k, v in out.items()}


_NC_CACHE = None


def _get_nc():
    global _NC_CACHE
    if _NC_CACHE is None:
        _NC_CACHE = build_program()
    return _NC_CACHE


def make_in_maps(inputs):
    w = prep_weights(inputs)
    x = np.ascontiguousarray(np.asarray(inputs["x"], np.float32))
    src = np.ascontiguousarray(np.asarray(inputs["source"], np.float32))
    in_maps = []
    for c in range(NCORES):
        b, ns = c // 4, (c % 4) * NCHUNK
        m = dict(w)
        m["x_chunk"] = np.ascontiguousarray(
            x[b].reshape(2, 128, N)[:, :, ns:ns + NCHUNK])
        m["source_b"] = np.ascontiguousarray(src[b].reshape(2, 128, M)).astype(
            __import__("ml_dtypes").bfloat16)
        in_maps.append(m)
    return in_maps


def assemble_out(results):
    out = np.empty((B, D, N), np.float32)
    for c in range(NCORES):
        b, ns = c // 4, (c % 4) * NCHUNK
        out[b].reshape(2, 128, N)[:, :, ns:ns + NCHUNK] = (
            results[c]["out_chunk"])
    return out


def kernel(**inputs):
    nc = _get_nc()
    res = bass_utils.run_bass_kernel_spmd(
        nc, make_in_maps(inputs), core_ids=list(range(NCORES)))
    return assemble_out(res.results)


# revision 15
# speedup vs baseline: 1.5980x; 1.2369x over previous
"""AttentionalPropagation (SuperGlue-style GNN message passing) on 8 trn2 cores.

Problem (hardcoded): B=2, D=256, N=M=4096, H=4 heads, head dim 64.
  q = P_q(x); k = P_k(source); v = P_v(source)      (bottleneck 1x1 convs D->D/8->D)
  msg = attn(q, k, v); merged = P_m(msg)            (per-head softmax over M)
  out = Conv(relu(BN(Conv(cat[x, merged]))))        (512->64->256)

Sharding: 8 cores = (batch b in {0,1}) x (query chunk of 1024).  Each core
computes k/v for its full batch row (cheap, duplicated 4x) and attention +
MLP for its 1024 query columns.  Weights replicated.  No collectives.

Layout: channels-on-partitions everywhere except attention scores, which are
computed transposed (keys m on partitions, queries n free) so softmax
normalization folds into the PE: the value matrix vT carries an extra
all-ones column per head, making row 64 of the msg-PSUM the softmax
denominator.  Head channels are made contiguous by permuting weight rows/cols
on the host.

Dtypes: attention path runs bf16 (error is attenuated: msg is a small additive
contribution vs x); the x -> MLP -> out path runs float32r.

HAM note: trn2's PE clock-gate only counts *full-K* (128-partition) matmuls as
activity; K<=64 matmuls run at 1.2 GHz forever.  So every hot matmul here is
padded to K=128 with zeros placed in the host-prepared weights (zero rows
contract against garbage-free operands), and the per-head scores matmul
contracts both heads' k against a zero-masked q.
"""

import numpy as np

import concourse.bass as bass
import concourse.mybir as mybir
import concourse.tile as tile
from concourse import bacc, bass_utils

B, D, N, M, H = 2, 256, 4096, 4096, 4
DIM = D // H       # 64
D8 = D // 8        # 32
TD = 2 * D         # 512
TD8 = TD // 8      # 64
BN_EPS = 1e-5
NCORES = 8
NCHUNK = N // 4    # query columns per core
NT = 512           # n tile (PSUM bank = 512 fp32)
NTILES = NCHUNK // NT          # 2
MT = 512           # source m tile for k/v projection stage
MTILES = M // MT               # 8
MC = 128           # m chunk (scores PSUM partition dim)
MCHUNKS = M // MC              # 32
BC = 2             # score chunks per exp batch (amortize ACT fixed cost)
NBATCH = MCHUNKS // BC
F32 = mybir.dt.float32
F32R = mybir.dt.float32r
BF16 = mybir.dt.bfloat16
AF = mybir.ActivationFunctionType


def _mm(nc, out, lhsT, rhs, start, stop):
    nc.tensor.matmul(out, lhsT, rhs, start=start, stop=stop)


def build_body(ctx, tc: tile.TileContext, io):
    nc = tc.nc
    x_d = io["x_chunk"]          # [2, 128, NCHUNK]  (channel-chunk, partition, n)
    src_d = io["source_b"]       # [2, 128, M]
    out_d = io["out_chunk"]      # [2, 128, NCHUNK]

    consts = ctx.enter_context(tc.tile_pool(name="consts", bufs=1))
    big = ctx.enter_context(tc.tile_pool(name="big", bufs=1))
    srcp = ctx.enter_context(tc.tile_pool(name="srcp", bufs=3))
    kv1p = ctx.enter_context(tc.tile_pool(name="kv1p", bufs=3))
    ep = ctx.enter_context(tc.tile_pool(name="ep", bufs=6))
    nrm = ctx.enter_context(tc.tile_pool(name="nrm", bufs=4))
    pp = ctx.enter_context(tc.tile_pool(name="pp", bufs=2, space="PSUM"))
    pps = ctx.enter_context(tc.tile_pool(name="pps", bufs=2, space="PSUM"))
    ppm = ctx.enter_context(tc.tile_pool(name="ppm", bufs=2, space="PSUM"))

    # ---- weights (host-preprocessed; zero-padded to K=128 where noted) ----
    def wtile(name, shape, dt=F32R):
        t = consts.tile(shape, dt, name=name)
        nc.sync.dma_start(out=t, in_=io[name])
        return t

    wq1t = wtile("wq1t", [128, 2, D8])            # f32r (x path)
    wk1t = wtile("wk1t", [128, 2, D8], BF16)
    wv1t = wtile("wv1t", [128, 2, D8], BF16)
    bq1 = wtile("bq1", [D8, 1], F32)
    bk1 = wtile("bk1", [D8, 1], F32)
    bv1 = wtile("bv1", [D8, 1], F32)
    wq2t = wtile("wq2t", [128, 2, 128], BF16)     # rows 0-31 W, 32 bias, rest 0
    wk2t = wtile("wk2t", [128, 2, 128], BF16)
    rvp = wtile("rvp", [128, H * 128], BF16)      # rows 64-95 Wv2'T, 96 bias/ones
    wm1t = wtile("wm1t", [128, H, D8], BF16)      # rows 64-127 zero
    bm1 = wtile("bm1", [1, D8])
    wm2t = wtile("wm2t", [128, 2, 128], BF16)
    wp1xt = wtile("wp1xt", [128, 2, TD8])         # f32r
    wp1mt = wtile("wp1mt", [128, 2, TD8], BF16)
    bp1 = wtile("bp1", [1, TD8])
    g1s = wtile("g1s", [TD8, 1], F32)
    be1 = wtile("be1", [TD8, 1], F32)
    wp2t = wtile("wp2t", [TD8 + 1, 2, 128])       # f32r
    ones_row = wtile("ones", [1, NCHUNK])

    # ---- persistent activations ----
    x_sb = big.tile([128, 2, NCHUNK], F32R)
    for ct in range(2):
        nc.sync.dma_start(out=x_sb[:, ct, :], in_=x_d[ct])
    k_sb = big.tile([128, 2, M], BF16)                 # k', heads contiguous
    vT_sb = big.tile([128, MCHUNKS, H * 128], BF16)    # [v'|ones|0pad] per head
    q_sb = big.tile([128, 2, NCHUNK], BF16)
    qz_sb = big.tile([128, 2, 2, NCHUNK], BF16)        # zero-masked q halves
    msg_sb = big.tile([128, H, NCHUNK], BF16)          # rows 64-127 zero
    nc.vector.memset(msg_sb[64:128, :, :], 0.0)

    # ---- k / v projections (full M, streamed in m tiles) ----
    # kv1: rows 0-31 k1, 32 ones, 33-63 zero, 64-95 v1, 96 ones, 97-127 zero
    for mt in range(MTILES):
        ms = mt * MT
        src = srcp.tile([128, 2, MT], BF16, tag="src")
        for ct in range(2):
            nc.sync.dma_start(out=src[:, ct, :], in_=src_d[ct, :, ms:ms + MT])
        kv1 = kv1p.tile([128, MT], BF16, tag="kv1")
        nc.vector.memset(kv1[32:64, :], 0.0)
        nc.vector.memset(kv1[96:128, :], 0.0)
        for (w1, b1, r0) in ((wk1t, bk1, 0), (wv1t, bv1, 64)):
            ps1 = pp.tile([D8, MT], F32, tag="pp", name="ps1")
            _mm(nc, ps1, w1[:, 0, :], src[:, 0, :], True, False)
            _mm(nc, ps1, w1[:, 1, :], src[:, 1, :], False, True)
            nc.vector.tensor_scalar_add(out=kv1[r0:r0 + D8, :], in0=ps1, scalar1=b1)
            nc.vector.tensor_copy(out=kv1[r0 + D8:r0 + D8 + 1, :],
                                  in_=ones_row[:, 0:MT])
        for ct in range(2):
            psk = pp.tile([128, MT], F32, tag="pp", name="psk")
            _mm(nc, psk, wk2t[:, ct, :], kv1, True, True)
            nc.vector.tensor_copy(out=k_sb[:, ct, ms:ms + MT], in_=psk)
        for j in range(MT // MC):
            mc = (ms // MC) + j
            psv = pp.tile([128, H * 128], F32, tag="pp", name="psv")
            _mm(nc, psv, kv1[:, j * MC:(j + 1) * MC], rvp, True, True)
            nc.vector.tensor_copy(out=vT_sb[:, mc, :], in_=psv)

    # ---- q projection (this core's n chunk) ----
    q1 = big.tile([128, NCHUNK], BF16)        # rows 0-31 q1, 32 ones, rest 0
    nc.vector.memset(q1[32:64, :], 0.0)
    nc.vector.memset(q1[64:128, :], 0.0)
    for nt in range(NTILES):
        ns = nt * NT
        psq = pp.tile([D8, NT], F32, tag="pp", name="psq")
        _mm(nc, psq, wq1t[:, 0, :], x_sb[:, 0, ns:ns + NT], True, False)
        _mm(nc, psq, wq1t[:, 1, :], x_sb[:, 1, ns:ns + NT], False, True)
        nc.vector.tensor_scalar_add(out=q1[0:D8, ns:ns + NT], in0=psq, scalar1=bq1)
    nc.vector.tensor_copy(out=q1[D8:D8 + 1, :], in_=ones_row)
    for ct in range(2):
        for nt in range(NTILES):
            ns = nt * NT
            psq2 = pp.tile([128, NT], F32, tag="pp", name="psq2")
            _mm(nc, psq2, wq2t[:, ct, :], q1[:, ns:ns + NT], True, True)
            nc.vector.tensor_copy(out=q_sb[:, ct, ns:ns + NT], in_=psq2)
    # zero-masked q: half 0 keeps rows 0-63, half 1 keeps rows 64-127
    nc.vector.memset(qz_sb[0:128, :, :, :], 0.0)
    for ct in range(2):
        nc.vector.tensor_copy(out=qz_sb[0:64, ct, 0, :], in_=q_sb[0:64, ct, :])
        nc.vector.tensor_copy(out=qz_sb[64:128, ct, 1, :], in_=q_sb[64:128, ct, :])

    # ---- attention ----
    # scores^T chunk [m=128, n=NT]: full-K matmul of both heads' k against the
    # zero-masked q of head h.  exp on ACT (scale folds 1/sqrt(DIM)), BC chunks
    # per instruction.  msg psum accumulates vT' @ exp; row 64 = denominator.
    for nt in range(NTILES):
        ns = nt * NT
        for h in range(H):
            ct, half = h // 2, h % 2
            pm = ppm.tile([128, NT], F32, tag="pm", name="pm")

            def emit_batch(bi):
                ps = pps.tile([128, BC, NT], F32, tag="ps", name="ps")
                for j in range(BC):
                    mc = bi * BC + j
                    _mm(nc, ps[:, j, :], k_sb[:, ct, mc * MC:(mc + 1) * MC],
                        qz_sb[:, ct, half, ns:ns + NT], True, True)
                e = ep.tile([128, BC, NT], BF16, tag="e", name="e")
                nc.scalar.activation(out=e, in_=ps, func=AF.Exp, scale=0.125)
                return e

            pend = emit_batch(0)
            for bi in range(NBATCH):
                nxt = emit_batch(bi + 1) if bi + 1 < NBATCH else None
                for j in range(BC):
                    mc = bi * BC + j
                    _mm(nc, pm, vT_sb[:, mc, h * 128:(h + 1) * 128],
                        pend[:, j, :], mc == 0, mc == MCHUNKS - 1)
                pend = nxt
            rec = nrm.tile([1, NT], F32, tag="rec", name="rec")
            nc.vector.reciprocal(out=rec, in_=pm[DIM:DIM + 1, :])
            bc = nrm.tile([DIM, NT], F32, tag="bc", name="bc")
            nc.gpsimd.partition_broadcast(bc, rec)
            nc.vector.tensor_mul(out=msg_sb[0:DIM, h, ns:ns + NT],
                                 in0=pm[0:DIM, :], in1=bc)

    # ---- merge projection ----
    m1 = big.tile([128, NCHUNK], BF16)        # rows 0-31 + ones row 32, rest 0
    nc.vector.memset(m1[32:64, :], 0.0)
    nc.vector.memset(m1[64:128, :], 0.0)
    for nt in range(NTILES):
        ns = nt * NT
        psm = pp.tile([D8, NT], F32, tag="pp", name="psm")
        for h in range(H):
            _mm(nc, psm, wm1t[:, h, :], msg_sb[:, h, ns:ns + NT], h == 0, False)
        _mm(nc, psm, bm1, ones_row[:, 0:NT], False, True)
        nc.vector.tensor_copy(out=m1[0:D8, ns:ns + NT], in_=psm)
    nc.vector.tensor_copy(out=m1[D8:D8 + 1, :], in_=ones_row)
    mm_sb = big.tile([128, 2, NCHUNK], BF16)      # merged msg, unpermuted chans
    for ct in range(2):
        for nt in range(NTILES):
            ns = nt * NT
            psm2 = pp.tile([128, NT], F32, tag="pp", name="psm2")
            _mm(nc, psm2, wm2t[:, ct, :], m1[:, ns:ns + NT], True, True)
            nc.vector.tensor_copy(out=mm_sb[:, ct, ns:ns + NT], in_=psm2)

    # ---- MLP: conv(cat[x, msg]) -> BN(eval) -> relu -> conv ----
    h1 = big.tile([TD8 + 1, NCHUNK], F32R)
    for nt in range(NTILES):
        ns = nt * NT
        psh = pp.tile([TD8, NT], F32, tag="pp", name="psh")
        _mm(nc, psh, wp1xt[:, 0, :], x_sb[:, 0, ns:ns + NT], True, False)
        _mm(nc, psh, wp1xt[:, 1, :], x_sb[:, 1, ns:ns + NT], False, False)
        _mm(nc, psh, wp1mt[:, 0, :], mm_sb[:, 0, ns:ns + NT], False, False)
        _mm(nc, psh, wp1mt[:, 1, :], mm_sb[:, 1, ns:ns + NT], False, False)
        _mm(nc, psh, bp1, ones_row[:, 0:NT], False, True)
        nc.scalar.activation(out=h1[0:TD8, ns:ns + NT], in_=psh, func=AF.Relu,
                             bias=be1, scale=g1s)
    nc.vector.tensor_copy(out=h1[TD8:TD8 + 1, :], in_=ones_row)
    out_sb = big.tile([128, 2, NCHUNK], F32)
    for ct in range(2):
        for nt in range(NTILES):
            ns = nt * NT
            pso = pp.tile([128, NT], F32, tag="pp", name="pso")
            _mm(nc, pso, wp2t[:, ct, :], h1[:, ns:ns + NT], True, True)
            nc.vector.tensor_copy(out=out_sb[:, ct, ns:ns + NT], in_=pso)
        nc.sync.dma_start(out=out_d[ct], in_=out_sb[:, ct, :])


def build_program():
    nc = bacc.Bacc("TRN2", target_bir_lowering=False, debug=False)
    io = {}
    def inp(name, shape, dt=F32R):
        io[name] = nc.dram_tensor(name, shape, dt, kind="ExternalInput").ap()
    inp("x_chunk", [2, 128, NCHUNK])
    inp("source_b", [2, 128, M], BF16)
    inp("wq1t", [128, 2, D8]); inp("bq1", [D8, 1], F32)
    inp("wk1t", [128, 2, D8], BF16); inp("bk1", [D8, 1], F32)
    inp("wv1t", [128, 2, D8], BF16); inp("bv1", [D8, 1], F32)
    inp("wq2t", [128, 2, 128], BF16)
    inp("wk2t", [128, 2, 128], BF16)
    inp("rvp", [128, H * 128], BF16)
    inp("wm1t", [128, H, D8], BF16); inp("bm1", [1, D8])
    inp("wm2t", [128, 2, 128], BF16)
    inp("wp1xt", [128, 2, TD8]); inp("wp1mt", [128, 2, TD8], BF16)
    inp("bp1", [1, TD8])
    inp("g1s", [TD8, 1], F32); inp("be1", [TD8, 1], F32)
    inp("wp2t", [TD8 + 1, 2, 128])
    inp("ones", [1, NCHUNK])
    io["out_chunk"] = nc.dram_tensor(
        "out_chunk", [2, 128, NCHUNK], F32, kind="ExternalOutput").ap()
    from contextlib import ExitStack
    with tile.TileContext(nc) as tc, ExitStack() as ctx:
        build_body(ctx, tc, io)
    nc.compile()
    return nc


def prep_weights(i):
    """Host-side preprocessing: transposes, head-channel permutation, bias
    folding (extra contraction rows), K=128 zero padding, BN folding."""
    import ml_dtypes
    bf = ml_dtypes.bfloat16
    f = np.float32
    a = {k: np.asarray(v, dtype=f) for k, v in i.items()}
    # permutation making head channels contiguous: c' = h*64+d  <- c = 4*d+h
    perm = (np.arange(H)[:, None] + H * np.arange(DIM)[None, :]).reshape(-1)

    def w1t(w):       # [D8, D] -> [128, 2, D8]
        return np.ascontiguousarray(w.T.reshape(2, 128, D8).swapaxes(0, 1))

    def w2tp(w, b):   # [D, D8] x [D] -> [128, 2, 128]: rows [w.T; b; zeros]
        o = np.zeros((128, 2, 128), f)
        o[0:D8] = w.T.reshape(D8, 2, 128)
        o[D8] = b.reshape(2, 128)
        return o

    out = {
        "wq1t": w1t(a["Wq1"]), "bq1": a["bq1"].reshape(D8, 1),
        "wk1t": w1t(a["Wk1"]), "bk1": a["bk1"].reshape(D8, 1),
        "wv1t": w1t(a["Wv1"]), "bv1": a["bv1"].reshape(D8, 1),
        "wq2t": w2tp(a["Wq2"][perm], a["bq2"][perm]),
        "wk2t": w2tp(a["Wk2"][perm], a["bk2"][perm]),
        "wm2t": w2tp(a["Wm2"], a["bm2"]),
        "wp2t": np.ascontiguousarray(np.concatenate(
            [a["Wp2"].T.reshape(TD8, 2, 128), a["bp2"].reshape(1, 2, 128)], 0)),
        "bm1": a["bm1"].reshape(1, D8),
        "bp1": a["bp1"].reshape(1, TD8),
        "g1s": (a["g1"] / np.sqrt(f(1.0) + f(BN_EPS))).reshape(TD8, 1).astype(f),
        "be1": a["be1"].reshape(TD8, 1),
        "ones": np.ones((1, NCHUNK), f),
    }
    # rvp [128, H*128]: kv1 layout has v1 at rows 64-95, ones at row 96.
    # per head h: cols [128h, 128h+64) = v weights; col 128h+64 = ones col
    # (softmax denominator); cols 128h+65.. zero.
    wv2p, bv2p = a["Wv2"][perm], a["bv2"][perm]
    rvp = np.zeros((128, H * 128), f)
    for h in range(H):
        c0 = h * 128
        rvp[64:64 + D8, c0:c0 + DIM] = wv2p[h * DIM:(h + 1) * DIM].T
        rvp[96, c0:c0 + DIM] = bv2p[h * DIM:(h + 1) * DIM]
        rvp[96, c0 + DIM] = 1.0
    out["rvp"] = rvp
    # wm1t [128, 4, D8]: [d, h, :] = Wm1'[:, h*64+d] for d<64, zeros below
    wm1p = a["Wm1"][:, perm]
    wm1t = np.zeros((128, H, D8), f)
    wm1t[0:DIM] = wm1p.T.reshape(H, DIM, D8).swapaxes(0, 1)
    out["wm1t"] = wm1t
    # mlp conv1 split into x-part and msg-part
    out["wp1xt"] = np.ascontiguousarray(
        a["Wp1"][:, 0:D].T.reshape(2, 128, TD8).swapaxes(0, 1))
    out["wp1mt"] = np.ascontiguousarray(
        a["Wp1"][:, D:TD].T.reshape(2, 128, TD8).swapaxes(0, 1))
    bf16_names = {"wk1t", "wv1t", "wq2t", "wk2t", "rvp", "wm1t", "wm2t", "wp1mt"}
    return {k: np.ascontiguousarray(v.astype(bf) if k in bf16_names else v)
            for k, v in out.items()}


_NC_CACHE = None


def _get_nc():
    global _NC_CACHE
    if _NC_CACHE is None:
        _NC_CACHE = build_program()
    return _NC_CACHE


def make_in_maps(inputs):
    import ml_dtypes
    w = prep_weights(inputs)
    x = np.ascontiguousarray(np.asarray(inputs["x"], np.float32))
    src = np.ascontiguousarray(np.asarray(inputs["source"], np.float32))
    in_maps = []
    for c in range(NCORES):
        b, ns = c // 4, (c % 4) * NCHUNK
        m = dict(w)
        m["x_chunk"] = np.ascontiguousarray(
            x[b].reshape(2, 128, N)[:, :, ns:ns + NCHUNK])
        m["source_b"] = np.ascontiguousarray(src[b].reshape(2, 128, M)).astype(
            ml_dtypes.bfloat16)
        in_maps.append(m)
    return in_maps


def assemble_out(results):
    out = np.empty((B, D, N), np.float32)
    for c in range(NCORES):
        b, ns = c // 4, (c % 4) * NCHUNK
        out[b].reshape(2, 128, N)[:, :, ns:ns + NCHUNK] = (
            results[c]["out_chunk"])
    return out


def kernel(**inputs):
    nc = _get_nc()
    res = bass_utils.run_bass_kernel_spmd(
        nc, make_in_maps(inputs), core_ids=list(range(NCORES)))
    return assemble_out(res.results)
